# revision 9
# baseline (speedup 1.0000x reference)
"""Trainium2 Bass kernel for the DualLoss nn.Module — v2.

Strategy (v2)
-------------
dist[b,m,s,n] = ||P[b,m,s] - X[b,n,m]||^2 via bf16 hi/lo split matmuls
(15 contraction rows per m: 9 coord-product rows + 3 pp + 3 xx splits).

BOTH passes now use K=120 block-diagonal stationaries (8 m's per group).
This keeps the PE's HAM clock gate warm (low-K matmuls run at 1.2 GHz,
K=120 runs at 2.4 GHz) and amortizes LDWEIGHTS.

  Pass A (d2, per (b,g,j)): PSUM[s=128, n=2048] = dist for m=8g+j.
    Moving operand is a zero-padded block-diagonal [120, 16384] SBUF
    buffer (zeroed once; diagonal blocks re-DMA'd per (b,g)).
    Drain: ACT stages n[0:1024] -> SBUF fp32; one TT_MINRED pairs it
    with PSUM n[1024:2048] and min-accumulates -> d2[s, m] directly.
  Pass B (d1, per (b,ch)): PSUM[n=128, (16m,128s)] = dist.
    Drain, two balanced modes (DVE vs ACT load):
      staged: ACT casts full tile -> SBUF bf16; DVE segmented
        tensor_reduce in 2x bf16 mode -> d1[n, 16m].
      TT:     ACT stages s[64:128] (strided) -> bf16; TT_MINRED pairs
        with PSUM s[0:64] -> bf16 pairwise mins; fat 2nd-level
        segmented reduce (batched over 2 tiles) -> d1.

Batch (B=16) is data-parallel across the 8 NeuronCores (2 batches/core).
Host applies argsort/stick-breaking + superquadric area weighting in f64.
"""

import sys

for _p in ("/opt/trn_rl_repo", "/root/.axon_site", "/root/.axon_site/_ro/trn_rl_repo",
           "/root/.axon_site/_ro/pypackages"):
    if _p not in sys.path:
        sys.path.append(_p)

import numpy as np

import concourse.bass as bass
import concourse.tile as tile
from concourse import bacc, mybir
from concourse.bass_utils import run_bass_kernel_spmd
from concourse import dve_ops as _dve_ops
from concourse.dve_ops import DveOp as _DveOp
from concourse.dve_spec import (
    Spec as _Spec, Src0 as _Src0, Src1 as _Src1, C0 as _C0, AluOp as _AluOp,
    minn as _minn, lower as _lower, _has_src1,
)
from concourse.dve_uop import DveOpSpec as _DveOpSpec


def _register_dve_op(name, spec):
    if name in _dve_ops._SUB_OPCODE_FOR_NAME:
        return next(op for op in _dve_ops.OPS if op.name == name)
    row = _dve_ops._CUSTOM_DVE_ROW_BASE + len(_dve_ops.OPS)
    assert row < 0x20
    _dve_ops._SUB_OPCODE_FOR_NAME[name] = row
    shas = {}
    for ver in ("v3", "v4"):
        tmp = _DveOpSpec(name=name, opcode=row, uops=_lower(spec, ver=ver),
                         rd1_en=_has_src1(spec))
        shas[ver] = tmp.sha(ver)
    op = _DveOp(name, spec, subdim=False, uops_sha=shas)
    _dve_ops.OPS.append(op)
    _dve_ops.CUSTOM_DVE_SPECS[name] = spec
    return op


# out = min(in0, in1); accum_out = min(s0, min(out))
TT_MINRED = _register_dve_op(
    "TT_MINRED_ANT",
    _Spec(
        body=_minn(_Src0, _Src1),
        accum=_AluOp.MIN,
        accum_init=_C0,
        reference=lambda in0, in1, s0, s1, imm2: np.minimum(
            in0.astype(np.float32), in1),
    ),
)

F32 = mybir.dt.float32
BF16 = mybir.dt.bfloat16
ALU = mybir.AluOpType

B, N, M, S = 16, 2048, 16, 128
CORES = 8
BPC = B // CORES          # 2
TPC = BPC * M             # 32
NCHUNK = N // 128         # 16
KR = 15
KK = 8 * KR               # 120
FOUR_PI = 4.0 * np.pi
BIG = 3.0e38

_PROGRAM = None
LAST_RESULTS = None


def _build_program():
    nc = bacc.Bacc("TRN2", target_bir_lowering=False, debug=False)

    a_stat_d = nc.dram_tensor("a_stat", [KK, 32, 128], BF16, kind="ExternalInput").ap()
    a_mov_d = nc.dram_tensor("a_mov", [4, KK, N], BF16, kind="ExternalInput").ap()
    b_stat_d = nc.dram_tensor("b_stat", [KK, TPC, 2, 128], BF16, kind="ExternalInput").ap()
    b_mov_d = nc.dram_tensor("b_mov", [KK, BPC, 2048], BF16, kind="ExternalInput").ap()
    d2a_d = nc.dram_tensor("d2a", [128, TPC], F32, kind="ExternalOutput").ap()
    d1o_d = nc.dram_tensor("d1o", [128, TPC, M], BF16, kind="ExternalOutput").ap()

    from contextlib import ExitStack

    with tile.TileContext(nc) as tc, ExitStack() as ctx:
        const = ctx.enter_context(tc.tile_pool(name="const", bufs=1))
        pool_psa = ctx.enter_context(tc.tile_pool(name="psa", bufs=2, space="PSUM"))
        pool_psb = ctx.enter_context(tc.tile_pool(name="psb", bufs=2, space="PSUM"))
        pool_stga = ctx.enter_context(tc.tile_pool(name="stga", bufs=4))
        pool_stgb = ctx.enter_context(tc.tile_pool(name="stgb", bufs=4))
        pool_stgs = ctx.enter_context(tc.tile_pool(name="stgs", bufs=3))
        pool_amov = ctx.enter_context(tc.tile_pool(name="amov", bufs=2))

        # resident inputs: only unit-0-critical chunks up front; the rest
        # trickle in inside the unit loop (see emit_late_dmas)
        a_stat = const.tile([KK, 32, 128], BF16)
        b_stat = const.tile([KK, TPC, 2, 128], BF16)
        b_mov = const.tile([KK, BPC, 2048], BF16)
        amov0 = pool_amov.tile([KK, N], BF16, tag="amov", name="amov0")
        nc.sync.dma_start(out=amov0[:, 0:512], in_=a_mov_d[0, :, 0:512])
        nc.sync.dma_start(out=a_stat[:, 0:8], in_=a_stat_d[:, 0:8])
        nc.sync.dma_start(out=amov0[:, 512:2048], in_=a_mov_d[0, :, 512:2048])
        nc.sync.dma_start(out=b_mov[:, 0, 0:1024], in_=b_mov_d[:, 0, 0:1024])
        nc.sync.dma_start(out=b_stat[:, 0:8], in_=b_stat_d[:, 0:8])
        nc.sync.dma_start(out=b_mov[:, 0, 1024:2048], in_=b_mov_d[:, 0, 1024:2048])

        def emit_late_dmas(u):
            # bulk constants on the idle GPSIMD (SWDGE) queue; chunked d1
            # output DMAs as soon as each 8-tile group is final
            if u == 2:
                nc.sync.dma_start(out=a_stat[:, 8:16], in_=a_stat_d[:, 8:16])
                nc.sync.dma_start(out=b_stat[:, 8:16], in_=b_stat_d[:, 8:16])
            elif u == 6:
                nc.sync.dma_start(out=a_stat[:, 16:24], in_=a_stat_d[:, 16:24])
                nc.sync.dma_start(out=b_stat[:, 16:24], in_=b_stat_d[:, 16:24])
                nc.sync.dma_start(out=b_mov[:, 1], in_=b_mov_d[:, 1])
            elif u == 10:
                nc.sync.dma_start(out=a_stat[:, 24:32], in_=a_stat_d[:, 24:32])
                nc.sync.dma_start(out=b_stat[:, 24:32], in_=b_stat_d[:, 24:32])
            if u % 8 == 7:
                k = u // 8
                nc.gpsimd.dma_start(out=d1o_d[:, 8 * k: 8 * k + 8],
                                    in_=d1all[:, 8 * k: 8 * k + 8])

        # outputs / scratch
        d2acc = const.tile([128, TPC], F32)
        d1all = const.tile([128, TPC, M], BF16)
        scr_a = const.tile([128, 1024], BF16)       # pass-A TT dump (unused data)
        junk = const.tile([128, 8], F32)
        # pass-B pairwise-min collector: 6 staged halves (3 tiles) per batch
        scr4 = const.tile([128, 6, 8, 64], BF16)
        scr5 = const.tile([128, 6, 8, 32], BF16)
        scr6 = const.tile([128, 6, 8, 16], BF16)
        scr7 = const.tile([128, 6, 8, 8], BF16)
        amov_cur = [None]
        nbhalf = [0]

        def emit_a_unit(u):
            bg, j = u // 8, u % 8
            if j == 0:
                if bg == 0:
                    amov_cur[0] = amov0
                else:
                    mv = pool_amov.tile([KK, N], BF16, tag="amov")
                    nc.sync.dma_start(out=mv[:], in_=a_mov_d[bg])
                    amov_cur[0] = mv
            mv = amov_cur[0]
            t1 = pool_psa.tile([128, 1024], F32, tag="psa")
            for q in range(2):
                nc.tensor.matmul(
                    t1[:, q * 512:(q + 1) * 512],
                    lhsT=a_stat[:, u, :],
                    rhs=mv[:, q * 512:(q + 1) * 512],
                    start=True, stop=True)
            stg = pool_stga.tile([128, 1024], F32, tag="stga")
            nc.scalar.copy(stg[:], t1[:])
            t2 = pool_psa.tile([128, 1024], F32, tag="psa")
            for q in range(2):
                nc.tensor.matmul(
                    t2[:, q * 512:(q + 1) * 512],
                    lhsT=a_stat[:, u, :],
                    rhs=mv[:, 1024 + q * 512: 1024 + (q + 1) * 512],
                    start=True, stop=True)
            nc.vector._custom_dve(
                TT_MINRED, out=scr_a[:], in0=t2[:], in1=stg[:],
                s0=BIG, accum_out=d2acc[:, u: u + 1])

        def emit_b_half(i, h):
            # PSUM half-tile [128, (2hs, 8mm, 64sw)]; columns reordered so the
            # two s-halves of each m are contiguous 512-blocks.
            b = i // NCHUNK
            ps = pool_psb.tile([128, 1024], F32, tag="psb")
            for q in range(2):
                nc.tensor.matmul(
                    ps[:, q * 512:(q + 1) * 512],
                    lhsT=b_stat[:, i, h, :],
                    rhs=b_mov[:, b, h * 1024 + q * 512: h * 1024 + (q + 1) * 512],
                    start=True, stop=True)
            if i % 4 == 3:
                # direct mode: segmented 1x reduce straight from PSUM
                nc.vector.tensor_reduce(
                    out=d1all[:, i, h * 8:(h + 1) * 8],
                    in_=ps[:].rearrange("p (hs m s) -> p m hs s", hs=2, m=8),
                    axis=mybir.AxisListType.XY, op=ALU.min)
                return
            # staged mode: ACT casts whole half -> bf16; TT bf16 2x L1
            stg = pool_stgb.tile([128, 1024], BF16, tag="stgb")
            nc.scalar.copy(stg[:], ps[:])
            k = nbhalf[0] % 6
            nc.vector.tensor_tensor(
                out=scr4[:, k], in0=stg[:, 0:512].rearrange(
                    "p (m s) -> p m s", m=8),
                in1=stg[:, 512:1024].rearrange("p (m s) -> p m s", m=8),
                op=ALU.min)
            nbhalf[0] += 1
            if nbhalf[0] % 6 == 0:
                # min-tree over 6 staged halves (= tiles i-2..i)
                nc.vector.tensor_tensor(
                    out=scr5[:], in0=scr4[:, :, :, 0:32],
                    in1=scr4[:, :, :, 32:64], op=ALU.min)
                nc.vector.tensor_tensor(
                    out=scr6[:], in0=scr5[:, :, :, 0:16],
                    in1=scr5[:, :, :, 16:32], op=ALU.min)
                nc.vector.tensor_tensor(
                    out=scr7[:], in0=scr6[:, :, :, 0:8],
                    in1=scr6[:, :, :, 8:16], op=ALU.min)
                i0 = i - 2
                nc.vector.tensor_reduce(
                    out=d1all[:, i0:i0 + 3, :],
                    in_=scr7[:].rearrange("p t m s -> p (t m) s"),
                    axis=mybir.AxisListType.X, op=ALU.min)

        # B runs two units behind A: startup B-input DMAs and the B drain
        # chains decouple further from A's critical path
        for u in range(TPC):
            emit_a_unit(u)
            if u >= 2:
                emit_b_half(u - 2, 0)
                emit_b_half(u - 2, 1)
                emit_late_dmas(u - 2)
        for ut in (TPC - 2, TPC - 1):
            emit_b_half(ut, 0)
            emit_b_half(ut, 1)
            emit_late_dmas(ut)

        for _k in range(4):
            nc.gpsimd.dma_start(out=d2a_d[:, 8 * _k: 8 * _k + 8],
                                in_=d2acc[:, 8 * _k: 8 * _k + 8])

    nc.compile()
    return nc


def _get_program():
    global _PROGRAM
    if _PROGRAM is None:
        _PROGRAM = _build_program()
    return _PROGRAM


def _make_in_maps(pcl, prim):
    import ml_dtypes
    bf = ml_dtypes.bfloat16
    Xf = np.asarray(pcl, np.float32)
    Pf = np.asarray(prim, np.float32)
    Xhi = Xf.astype(bf).astype(np.float32)
    Xlo = (Xf - Xhi).astype(bf).astype(np.float32)
    Phi = Pf.astype(bf).astype(np.float32)
    Plo = (Pf - Phi).astype(bf).astype(np.float32)
    X64 = Xhi.astype(np.float64) + Xlo
    P64 = Phi.astype(np.float64) + Plo
    xx64 = np.einsum("bnmc,bnmc->bnm", X64, X64)
    pp64 = np.einsum("bmsc,bmsc->bms", P64, P64)

    def split3(v64):
        b0 = v64.astype(np.float32).astype(bf).astype(np.float64)
        r1 = v64 - b0
        b1 = r1.astype(np.float32).astype(bf).astype(np.float64)
        b2 = (r1 - b1).astype(np.float32).astype(bf).astype(np.float64)
        return np.stack([b0, b1, b2]).astype(np.float32)

    xx_b = split3(xx64)                                    # (3, B, N, M)
    pp_b = split3(pp64)                                    # (3, B, M, S)

    XhiT = Xhi.transpose(0, 2, 3, 1)                       # (B, M, 3, N)
    XloT = Xlo.transpose(0, 2, 3, 1)
    PhiS = Phi.transpose(0, 1, 3, 2)                       # (B, M, 3, S)
    PloS = Plo.transpose(0, 1, 3, 2)

    # ---- pass A ----
    a_stat_all = np.empty((B, M, KR, S), np.float32)
    a_stat_all[:, :, 0:3] = -2.0 * PhiS
    a_stat_all[:, :, 3:6] = -2.0 * PhiS
    a_stat_all[:, :, 6:9] = -2.0 * PloS
    a_stat_all[:, :, 9:12] = pp_b.transpose(1, 2, 0, 3)
    a_stat_all[:, :, 12:15] = 1.0

    a_movc_all = np.empty((B, M, KR, N), np.float32)
    xxT = xx_b.transpose(1, 3, 0, 2)                       # (B, M, 3, N)
    a_movc_all[:, :, 0:3] = XhiT
    a_movc_all[:, :, 3:6] = XloT
    a_movc_all[:, :, 6:9] = XhiT
    a_movc_all[:, :, 9:12] = 1.0
    a_movc_all[:, :, 12:15] = xxT

    # ---- pass B (identical to baseline) ----
    b_stat_all = np.empty((B, M, KR, N), np.float32)
    b_stat_all[:, :, 0:3] = -2.0 * XhiT
    b_stat_all[:, :, 3:6] = -2.0 * XhiT
    b_stat_all[:, :, 6:9] = -2.0 * XloT
    b_stat_all[:, :, 9:12] = 1.0
    b_stat_all[:, :, 12:15] = xx_b.transpose(1, 3, 0, 2)
    b_stat_all = b_stat_all.reshape(B, 2, KK, NCHUNK, 128)
    b_mov_all = np.zeros((B, KK, M * S), np.float32)
    for m in range(M):
        r0 = KR * (m % 8)
        h = m // 8
        # column order within half h: c = h*1024 + hs*512 + (m%8)*64 + sw
        for hs in range(2):
            cs = slice(h * 1024 + hs * 512 + (m % 8) * 64,
                       h * 1024 + hs * 512 + (m % 8) * 64 + 64)
            ss = slice(hs * 64, hs * 64 + 64)
            b_mov_all[:, r0 + 0: r0 + 3, cs] = PhiS[:, m, :, ss]
            b_mov_all[:, r0 + 3: r0 + 6, cs] = PloS[:, m, :, ss]
            b_mov_all[:, r0 + 6: r0 + 9, cs] = PhiS[:, m, :, ss]
            b_mov_all[:, r0 + 9: r0 + 12, cs] = pp_b[:, :, m].transpose(1, 0, 2)[:, :, ss]
            b_mov_all[:, r0 + 12: r0 + 15, cs] = 1.0

    in_maps = []
    for c in range(CORES):
        sl = slice(BPC * c, BPC * (c + 1))
        # a_stat: per unit u=(b,g,j): [120, 128] zero except rows 15j:15j+15
        asrc = a_stat_all[sl].reshape(2, 2, 8, KR, S)      # (b, g, j, r, s)
        ast = np.zeros((KK, 32, S), np.float32)
        for u in range(32):
            bq, g, j = u // 16, (u // 8) % 2, u % 8
            ast[KR * j: KR * j + KR, u] = asrc[bq, g, j]
        # a_mov: per (b,g): dense [120, 2048] — band j holds m=8g+j's rows
        amv = (a_movc_all[sl].reshape(2, 2, 8, KR, N)
               .reshape(4, KK, N))
        in_maps.append({
            "a_stat": np.ascontiguousarray(ast).astype(bf),
            "a_mov": np.ascontiguousarray(amv).astype(bf),
            "b_stat": np.ascontiguousarray(
                b_stat_all[sl].transpose(2, 0, 3, 1, 4).reshape(KK, TPC, 2, 128)).astype(bf),
            "b_mov": np.ascontiguousarray(
                b_mov_all[sl].transpose(1, 0, 2)).astype(bf),
        })
    return in_maps


def kernel(pcl_transformed, primitive_points, size, probs, _trace=False):
    global LAST_RESULTS
    pcl = np.asarray(pcl_transformed, dtype=np.float32)
    prim = np.asarray(primitive_points, dtype=np.float32)
    size = np.asarray(size, dtype=np.float32)
    probs = np.asarray(probs, dtype=np.float32)

    nc = _get_program()
    in_maps = _make_in_maps(pcl, prim)
    res = run_bass_kernel_spmd(nc, in_maps, list(range(CORES)), trace=_trace)
    LAST_RESULTS = res

    d2min = np.empty((B, M, S), np.float64)
    d1 = np.empty((B, N, M), np.float64)
    for c in range(CORES):
        d2a = res.results[c]["d2a"].astype(np.float64)       # [128(s), 32]
        d2min[BPC * c: BPC * (c + 1)] = d2a.T.reshape(BPC, M, S)
        d1o = res.results[c]["d1o"].astype(np.float64)       # [128, TPC, M] (bf16)
        d1[BPC * c: BPC * (c + 1)] = (
            d1o.reshape(128, BPC, NCHUNK, M).transpose(1, 2, 0, 3)
            .reshape(BPC, N, M))

    p64v = probs.astype(np.float64)
    d1f = d1.reshape(B * N, M)
    order = np.argsort(d1f, axis=1, kind="stable")
    ps = np.take_along_axis(np.repeat(p64v, N, axis=0), order, axis=1)
    ncp = np.cumprod(1.0 - ps, axis=1)
    ncp = np.concatenate([np.ones((B * N, 1)), ncp[:, :-1]], axis=1)
    p2p_sum = float((np.take_along_axis(d1f, order, axis=1) * ps * ncp).sum())

    d2 = np.where(d2min >= 1e30, 0.0, d2min)

    s0 = size[..., 0].astype(np.float64)
    s1 = size[..., 1].astype(np.float64)
    s2 = size[..., 2].astype(np.float64)
    area = FOUR_PI * ((s0 * s1) ** 1.6 / 3 + (s0 * s2) ** 1.6 / 3
                      + (s1 * s2) ** 1.6 / 3) ** 0.625
    area = M * area / area.sum(axis=-1, keepdims=True)

    prim_to_pcl = float(
        (d2.mean(axis=-1) * probs.astype(np.float64) * area).sum() / (B * M))
    pcl_to_prim = float(p2p_sum / (B * N))

    total = np.float32(pcl_to_prim + prim_to_pcl)
    return (total,
            np.float32(pcl_to_prim),
            np.float32(prim_to_pcl),
            np.float32(0.0))


# revision 10
# speedup vs baseline: 1.0145x; 1.0145x over previous
"""Trainium2 Bass kernel for the DualLoss nn.Module — v2.

Strategy (v2)
-------------
dist[b,m,s,n] = ||P[b,m,s] - X[b,n,m]||^2 via bf16 hi/lo split matmuls
(15 contraction rows per m: 9 coord-product rows + 3 pp + 3 xx splits).

BOTH passes now use K=120 block-diagonal stationaries (8 m's per group).
This keeps the PE's HAM clock gate warm (low-K matmuls run at 1.2 GHz,
K=120 runs at 2.4 GHz) and amortizes LDWEIGHTS.

  Pass A (d2, per (b,g,j)): PSUM[s=128, n=2048] = dist for m=8g+j.
    Moving operand is a zero-padded block-diagonal [120, 16384] SBUF
    buffer (zeroed once; diagonal blocks re-DMA'd per (b,g)).
    Drain: ACT stages n[0:1024] -> SBUF fp32; one TT_MINRED pairs it
    with PSUM n[1024:2048] and min-accumulates -> d2[s, m] directly.
  Pass B (d1, per (b,ch)): PSUM[n=128, (16m,128s)] = dist.
    Drain, two balanced modes (DVE vs ACT load):
      staged: ACT casts full tile -> SBUF bf16; DVE segmented
        tensor_reduce in 2x bf16 mode -> d1[n, 16m].
      TT:     ACT stages s[64:128] (strided) -> bf16; TT_MINRED pairs
        with PSUM s[0:64] -> bf16 pairwise mins; fat 2nd-level
        segmented reduce (batched over 2 tiles) -> d1.

Batch (B=16) is data-parallel across the 8 NeuronCores (2 batches/core).
Host applies argsort/stick-breaking + superquadric area weighting in f64.
"""

import sys

for _p in ("/opt/trn_rl_repo", "/root/.axon_site", "/root/.axon_site/_ro/trn_rl_repo",
           "/root/.axon_site/_ro/pypackages"):
    if _p not in sys.path:
        sys.path.append(_p)

import numpy as np

import concourse.bass as bass
import concourse.tile as tile
from concourse import bacc, mybir
from concourse.bass_utils import run_bass_kernel_spmd
from concourse import dve_ops as _dve_ops
from concourse.dve_ops import DveOp as _DveOp
from concourse.dve_spec import (
    Spec as _Spec, Src0 as _Src0, Src1 as _Src1, C0 as _C0, AluOp as _AluOp,
    minn as _minn, lower as _lower, _has_src1,
)
from concourse.dve_uop import DveOpSpec as _DveOpSpec


def _register_dve_op(name, spec):
    if name in _dve_ops._SUB_OPCODE_FOR_NAME:
        return next(op for op in _dve_ops.OPS if op.name == name)
    row = _dve_ops._CUSTOM_DVE_ROW_BASE + len(_dve_ops.OPS)
    assert row < 0x20
    _dve_ops._SUB_OPCODE_FOR_NAME[name] = row
    shas = {}
    for ver in ("v3", "v4"):
        tmp = _DveOpSpec(name=name, opcode=row, uops=_lower(spec, ver=ver),
                         rd1_en=_has_src1(spec))
        shas[ver] = tmp.sha(ver)
    op = _DveOp(name, spec, subdim=False, uops_sha=shas)
    _dve_ops.OPS.append(op)
    _dve_ops.CUSTOM_DVE_SPECS[name] = spec
    return op


# out = min(in0, in1); accum_out = min(s0, min(out))
TT_MINRED = _register_dve_op(
    "TT_MINRED_ANT",
    _Spec(
        body=_minn(_Src0, _Src1),
        accum=_AluOp.MIN,
        accum_init=_C0,
        reference=lambda in0, in1, s0, s1, imm2: np.minimum(
            in0.astype(np.float32), in1),
    ),
)

F32 = mybir.dt.float32
BF16 = mybir.dt.bfloat16
ALU = mybir.AluOpType

B, N, M, S = 16, 2048, 16, 128
CORES = 8
BPC = B // CORES          # 2
TPC = BPC * M             # 32
NCHUNK = N // 128         # 16
KR = 15
KK = 8 * KR               # 120
FOUR_PI = 4.0 * np.pi
BIG = 3.0e38

_PROGRAM = None
LAST_RESULTS = None


def _build_program():
    nc = bacc.Bacc("TRN2", target_bir_lowering=False, debug=False)

    a_stat_d = nc.dram_tensor("a_stat", [KK, 32, 128], BF16, kind="ExternalInput").ap()
    a_mov_d = nc.dram_tensor("a_mov", [4, KK, N], BF16, kind="ExternalInput").ap()
    b_stat_d = nc.dram_tensor("b_stat", [KK, TPC, 2, 128], BF16, kind="ExternalInput").ap()
    b_mov_d = nc.dram_tensor("b_mov", [KK, BPC, 2048], BF16, kind="ExternalInput").ap()
    d2a_d = nc.dram_tensor("d2a", [128, TPC], F32, kind="ExternalOutput").ap()
    d1o_d = nc.dram_tensor("d1o", [128, TPC, M], BF16, kind="ExternalOutput").ap()

    from contextlib import ExitStack

    with tile.TileContext(nc) as tc, ExitStack() as ctx:
        const = ctx.enter_context(tc.tile_pool(name="const", bufs=1))
        pool_ps = ctx.enter_context(tc.tile_pool(name="ps", bufs=4, space="PSUM"))
        pool_stga = ctx.enter_context(tc.tile_pool(name="stga", bufs=4))
        pool_stgb = ctx.enter_context(tc.tile_pool(name="stgb", bufs=4))
        pool_stgs = ctx.enter_context(tc.tile_pool(name="stgs", bufs=3))
        pool_amov = ctx.enter_context(tc.tile_pool(name="amov", bufs=2))

        # resident inputs: only unit-0-critical chunks up front; the rest
        # trickle in inside the unit loop (see emit_late_dmas)
        a_stat = const.tile([KK, 32, 128], BF16)
        b_stat = const.tile([KK, TPC, 2, 128], BF16)
        b_mov = const.tile([KK, BPC, 2048], BF16)
        amov0 = pool_amov.tile([KK, N], BF16, tag="amov", name="amov0")
        nc.sync.dma_start(out=amov0[:, 0:512], in_=a_mov_d[0, :, 0:512])
        nc.sync.dma_start(out=a_stat[:, 0:8], in_=a_stat_d[:, 0:8])
        nc.sync.dma_start(out=amov0[:, 512:2048], in_=a_mov_d[0, :, 512:2048])
        nc.sync.dma_start(out=b_mov[:, 0, 0:1024], in_=b_mov_d[:, 0, 0:1024])
        nc.sync.dma_start(out=b_stat[:, 0:8], in_=b_stat_d[:, 0:8])
        nc.sync.dma_start(out=b_mov[:, 0, 1024:2048], in_=b_mov_d[:, 0, 1024:2048])

        def emit_late_dmas(u):
            # bulk constants on the idle GPSIMD (SWDGE) queue; chunked d1
            # output DMAs as soon as each 8-tile group is final
            if u == 2:
                nc.sync.dma_start(out=a_stat[:, 8:16], in_=a_stat_d[:, 8:16])
                nc.sync.dma_start(out=b_stat[:, 8:16], in_=b_stat_d[:, 8:16])
            elif u == 6:
                nc.sync.dma_start(out=a_stat[:, 16:24], in_=a_stat_d[:, 16:24])
                nc.sync.dma_start(out=b_stat[:, 16:24], in_=b_stat_d[:, 16:24])
                nc.sync.dma_start(out=b_mov[:, 1], in_=b_mov_d[:, 1])
            elif u == 10:
                nc.sync.dma_start(out=a_stat[:, 24:32], in_=a_stat_d[:, 24:32])
                nc.sync.dma_start(out=b_stat[:, 24:32], in_=b_stat_d[:, 24:32])
            if u % 8 == 7:
                k = u // 8
                nc.gpsimd.dma_start(out=d1o_d[:, 8 * k: 8 * k + 8],
                                    in_=d1all[:, 8 * k: 8 * k + 8])

        # outputs / scratch
        d2acc = const.tile([128, TPC], F32)
        d1all = const.tile([128, TPC, M], BF16)
        scr_a = const.tile([128, 1024], BF16)       # pass-A TT dump (unused data)
        junk = const.tile([128, 8], F32)
        # pass-B pairwise-min collector: 6 staged halves (3 tiles) per batch
        scr4 = const.tile([128, 6, 8, 64], BF16)
        scr5 = const.tile([128, 6, 8, 32], BF16)
        scr6 = const.tile([128, 6, 8, 16], BF16)
        scr7 = const.tile([128, 6, 8, 8], BF16)
        amov_cur = [None]
        nbhalf = [0]

        def emit_a_unit(u):
            bg, j = u // 8, u % 8
            if j == 0:
                if bg == 0:
                    amov_cur[0] = amov0
                else:
                    mv = pool_amov.tile([KK, N], BF16, tag="amov")
                    nc.sync.dma_start(out=mv[:], in_=a_mov_d[bg])
                    amov_cur[0] = mv
            mv = amov_cur[0]
            t1 = pool_ps.tile([128, 1024], F32, tag="ps")
            for q in range(2):
                nc.tensor.matmul(
                    t1[:, q * 512:(q + 1) * 512],
                    lhsT=a_stat[:, u, :],
                    rhs=mv[:, q * 512:(q + 1) * 512],
                    start=True, stop=True)
            stg = pool_stga.tile([128, 1024], F32, tag="stga")
            nc.scalar.copy(stg[:], t1[:])
            t2 = pool_ps.tile([128, 1024], F32, tag="ps")
            for q in range(2):
                nc.tensor.matmul(
                    t2[:, q * 512:(q + 1) * 512],
                    lhsT=a_stat[:, u, :],
                    rhs=mv[:, 1024 + q * 512: 1024 + (q + 1) * 512],
                    start=True, stop=True)
            nc.vector._custom_dve(
                TT_MINRED, out=scr_a[:], in0=t2[:], in1=stg[:],
                s0=BIG, accum_out=d2acc[:, u: u + 1])

        def emit_b_half(i, h):
            # PSUM half-tile [128, (2hs, 8mm, 64sw)]; columns reordered so the
            # two s-halves of each m are contiguous 512-blocks.
            b = i // NCHUNK
            ps = pool_ps.tile([128, 1024], F32, tag="ps")
            for q in range(2):
                nc.tensor.matmul(
                    ps[:, q * 512:(q + 1) * 512],
                    lhsT=b_stat[:, i, h, :],
                    rhs=b_mov[:, b, h * 1024 + q * 512: h * 1024 + (q + 1) * 512],
                    start=True, stop=True)
            if i % 4 == 3:
                # direct mode: segmented 1x reduce straight from PSUM
                nc.vector.tensor_reduce(
                    out=d1all[:, i, h * 8:(h + 1) * 8],
                    in_=ps[:].rearrange("p (hs m s) -> p m hs s", hs=2, m=8),
                    axis=mybir.AxisListType.XY, op=ALU.min)
                return
            # staged mode: ACT casts whole half -> bf16; TT bf16 2x L1
            stg = pool_stgb.tile([128, 1024], BF16, tag="stgb")
            nc.scalar.copy(stg[:], ps[:])
            k = nbhalf[0] % 6
            nc.vector.tensor_tensor(
                out=scr4[:, k], in0=stg[:, 0:512].rearrange(
                    "p (m s) -> p m s", m=8),
                in1=stg[:, 512:1024].rearrange("p (m s) -> p m s", m=8),
                op=ALU.min)
            nbhalf[0] += 1
            if nbhalf[0] % 6 == 0:
                # min-tree over 6 staged halves (= tiles i-2..i)
                nc.vector.tensor_tensor(
                    out=scr5[:], in0=scr4[:, :, :, 0:32],
                    in1=scr4[:, :, :, 32:64], op=ALU.min)
                nc.vector.tensor_tensor(
                    out=scr6[:], in0=scr5[:, :, :, 0:16],
                    in1=scr5[:, :, :, 16:32], op=ALU.min)
                nc.vector.tensor_tensor(
                    out=scr7[:], in0=scr6[:, :, :, 0:8],
                    in1=scr6[:, :, :, 8:16], op=ALU.min)
                i0 = i - 2
                nc.vector.tensor_reduce(
                    out=d1all[:, i0:i0 + 3, :],
                    in_=scr7[:].rearrange("p t m s -> p (t m) s"),
                    axis=mybir.AxisListType.X, op=ALU.min)

        # B runs two units behind A: startup B-input DMAs and the B drain
        # chains decouple further from A's critical path
        for u in range(TPC):
            emit_a_unit(u)
            if u >= 2:
                emit_b_half(u - 2, 0)
                emit_b_half(u - 2, 1)
                emit_late_dmas(u - 2)
        for ut in (TPC - 2, TPC - 1):
            emit_b_half(ut, 0)
            emit_b_half(ut, 1)
            emit_late_dmas(ut)

        for _k in range(4):
            nc.gpsimd.dma_start(out=d2a_d[:, 8 * _k: 8 * _k + 8],
                                in_=d2acc[:, 8 * _k: 8 * _k + 8])

    nc.compile()
    return nc


def _get_program():
    global _PROGRAM
    if _PROGRAM is None:
        _PROGRAM = _build_program()
    return _PROGRAM


def _make_in_maps(pcl, prim):
    import ml_dtypes
    bf = ml_dtypes.bfloat16
    Xf = np.asarray(pcl, np.float32)
    Pf = np.asarray(prim, np.float32)
    Xhi = Xf.astype(bf).astype(np.float32)
    Xlo = (Xf - Xhi).astype(bf).astype(np.float32)
    Phi = Pf.astype(bf).astype(np.float32)
    Plo = (Pf - Phi).astype(bf).astype(np.float32)
    X64 = Xhi.astype(np.float64) + Xlo
    P64 = Phi.astype(np.float64) + Plo
    xx64 = np.einsum("bnmc,bnmc->bnm", X64, X64)
    pp64 = np.einsum("bmsc,bmsc->bms", P64, P64)

    def split3(v64):
        b0 = v64.astype(np.float32).astype(bf).astype(np.float64)
        r1 = v64 - b0
        b1 = r1.astype(np.float32).astype(bf).astype(np.float64)
        b2 = (r1 - b1).astype(np.float32).astype(bf).astype(np.float64)
        return np.stack([b0, b1, b2]).astype(np.float32)

    xx_b = split3(xx64)                                    # (3, B, N, M)
    pp_b = split3(pp64)                                    # (3, B, M, S)

    XhiT = Xhi.transpose(0, 2, 3, 1)                       # (B, M, 3, N)
    XloT = Xlo.transpose(0, 2, 3, 1)
    PhiS = Phi.transpose(0, 1, 3, 2)                       # (B, M, 3, S)
    PloS = Plo.transpose(0, 1, 3, 2)

    # ---- pass A ----
    a_stat_all = np.empty((B, M, KR, S), np.float32)
    a_stat_all[:, :, 0:3] = -2.0 * PhiS
    a_stat_all[:, :, 3:6] = -2.0 * PhiS
    a_stat_all[:, :, 6:9] = -2.0 * PloS
    a_stat_all[:, :, 9:12] = pp_b.transpose(1, 2, 0, 3)
    a_stat_all[:, :, 12:15] = 1.0

    a_movc_all = np.empty((B, M, KR, N), np.float32)
    xxT = xx_b.transpose(1, 3, 0, 2)                       # (B, M, 3, N)
    a_movc_all[:, :, 0:3] = XhiT
    a_movc_all[:, :, 3:6] = XloT
    a_movc_all[:, :, 6:9] = XhiT
    a_movc_all[:, :, 9:12] = 1.0
    a_movc_all[:, :, 12:15] = xxT

    # ---- pass B (identical to baseline) ----
    b_stat_all = np.empty((B, M, KR, N), np.float32)
    b_stat_all[:, :, 0:3] = -2.0 * XhiT
    b_stat_all[:, :, 3:6] = -2.0 * XhiT
    b_stat_all[:, :, 6:9] = -2.0 * XloT
    b_stat_all[:, :, 9:12] = 1.0
    b_stat_all[:, :, 12:15] = xx_b.transpose(1, 3, 0, 2)
    b_stat_all = b_stat_all.reshape(B, 2, KK, NCHUNK, 128)
    b_mov_all = np.zeros((B, KK, M * S), np.float32)
    for m in range(M):
        r0 = KR * (m % 8)
        h = m // 8
        # column order within half h: c = h*1024 + hs*512 + (m%8)*64 + sw
        for hs in range(2):
            cs = slice(h * 1024 + hs * 512 + (m % 8) * 64,
                       h * 1024 + hs * 512 + (m % 8) * 64 + 64)
            ss = slice(hs * 64, hs * 64 + 64)
            b_mov_all[:, r0 + 0: r0 + 3, cs] = PhiS[:, m, :, ss]
            b_mov_all[:, r0 + 3: r0 + 6, cs] = PloS[:, m, :, ss]
            b_mov_all[:, r0 + 6: r0 + 9, cs] = PhiS[:, m, :, ss]
            b_mov_all[:, r0 + 9: r0 + 12, cs] = pp_b[:, :, m].transpose(1, 0, 2)[:, :, ss]
            b_mov_all[:, r0 + 12: r0 + 15, cs] = 1.0

    in_maps = []
    for c in range(CORES):
        sl = slice(BPC * c, BPC * (c + 1))
        # a_stat: per unit u=(b,g,j): [120, 128] zero except rows 15j:15j+15
        asrc = a_stat_all[sl].reshape(2, 2, 8, KR, S)      # (b, g, j, r, s)
        ast = np.zeros((KK, 32, S), np.float32)
        for u in range(32):
            bq, g, j = u // 16, (u // 8) % 2, u % 8
            ast[KR * j: KR * j + KR, u] = asrc[bq, g, j]
        # a_mov: per (b,g): dense [120, 2048] — band j holds m=8g+j's rows
        amv = (a_movc_all[sl].reshape(2, 2, 8, KR, N)
               .reshape(4, KK, N))
        in_maps.append({
            "a_stat": np.ascontiguousarray(ast).astype(bf),
            "a_mov": np.ascontiguousarray(amv).astype(bf),
            "b_stat": np.ascontiguousarray(
                b_stat_all[sl].transpose(2, 0, 3, 1, 4).reshape(KK, TPC, 2, 128)).astype(bf),
            "b_mov": np.ascontiguousarray(
                b_mov_all[sl].transpose(1, 0, 2)).astype(bf),
        })
    return in_maps


def kernel(pcl_transformed, primitive_points, size, probs, _trace=False):
    global LAST_RESULTS
    pcl = np.asarray(pcl_transformed, dtype=np.float32)
    prim = np.asarray(primitive_points, dtype=np.float32)
    size = np.asarray(size, dtype=np.float32)
    probs = np.asarray(probs, dtype=np.float32)

    nc = _get_program()
    in_maps = _make_in_maps(pcl, prim)
    res = run_bass_kernel_spmd(nc, in_maps, list(range(CORES)), trace=_trace)
    LAST_RESULTS = res

    d2min = np.empty((B, M, S), np.float64)
    d1 = np.empty((B, N, M), np.float64)
    for c in range(CORES):
        d2a = res.results[c]["d2a"].astype(np.float64)       # [128(s), 32]
        d2min[BPC * c: BPC * (c + 1)] = d2a.T.reshape(BPC, M, S)
        d1o = res.results[c]["d1o"].astype(np.float64)       # [128, TPC, M] (bf16)
        d1[BPC * c: BPC * (c + 1)] = (
            d1o.reshape(128, BPC, NCHUNK, M).transpose(1, 2, 0, 3)
            .reshape(BPC, N, M))

    p64v = probs.astype(np.float64)
    d1f = d1.reshape(B * N, M)
    order = np.argsort(d1f, axis=1, kind="stable")
    ps = np.take_along_axis(np.repeat(p64v, N, axis=0), order, axis=1)
    ncp = np.cumprod(1.0 - ps, axis=1)
    ncp = np.concatenate([np.ones((B * N, 1)), ncp[:, :-1]], axis=1)
    p2p_sum = float((np.take_along_axis(d1f, order, axis=1) * ps * ncp).sum())

    d2 = np.where(d2min >= 1e30, 0.0, d2min)

    s0 = size[..., 0].astype(np.float64)
    s1 = size[..., 1].astype(np.float64)
    s2 = size[..., 2].astype(np.float64)
    area = FOUR_PI * ((s0 * s1) ** 1.6 / 3 + (s0 * s2) ** 1.6 / 3
                      + (s1 * s2) ** 1.6 / 3) ** 0.625
    area = M * area / area.sum(axis=-1, keepdims=True)

    prim_to_pcl = float(
        (d2.mean(axis=-1) * probs.astype(np.float64) * area).sum() / (B * M))
    pcl_to_prim = float(p2p_sum / (B * N))

    total = np.float32(pcl_to_prim + prim_to_pcl)
    return (total,
            np.float32(pcl_to_prim),
            np.float32(prim_to_pcl),
            np.float32(0.0))
